# revision 10
# baseline (speedup 1.0000x reference)
"""Trainium2 Bass kernel: attention block (QKV + RoPE + ALiBi attention + proj).

Problem shapes: x [2, 2048, 1024], 16 heads x 64 dim, f32 I/O.
Sharding: batch (2) x head-groups (4 heads/core) = 8 cores. Per-slot
AllToAlls exchange per-head outputs; each core gathers its own
query-quarter columns via indirect DMA and runs the output projection.

v2 schedule: the whole kernel is one continuous 128x128-mode PE stream.
Attention is Scalar(exp)-paced, so the remaining QKV work (k23/q23),
v-compute, and the first projection half are interleaved as PE filler
inside the attention j-loops. RoPE tables are shared q/k bf16 (SCALE is
folded into the exp activation's scale argument). Projection runs in two
passes (slots 0/1 as filler inside attn2/3, slots 2/3 in the tail) with
an SBUF f32 accumulator.

Dataflow (transposed on-chip):
  xT [C, N] --QKV--> qT/kT [D, N] (RoPE'd, d pairwise-interleaved) and v [N, D]
  sT = bd(kT).T @ qd  [j, i] -> p = exp(SCALE*sT) * ebias  (no max-sub)
  oT = v_aug.T @ p    [d+1, i] (ones column gives softmax denominators)
  yT = oT[:64] * (1/denom broadcast) -> AllToAll -> gather -> proj.
ALiBi is applied multiplicatively (exp(slope*min(j-i,0)) Toeplitz tables);
far-past tiles below ~1e-3 relative mass are skipped per-slot (SLOT_CUT).
"""
import sys
if "/opt/trn_rl_repo" not in sys.path:
    sys.path.insert(0, "/opt/trn_rl_repo")

import numpy as np
import ml_dtypes

import concourse.bass as bass
import concourse.mybir as mybir
import concourse.tile as tile
from concourse import bacc
from concourse.bass_utils import run_bass_kernel_spmd

BF = mybir.dt.bfloat16
F32 = mybir.dt.float32
NPBF = ml_dtypes.bfloat16

B, N, C = 2, 2048, 1024
H, D = 16, 64
G = 4                       # heads per core
N_CORES = 8
MAX_BIAS = 8.0
SCALE = D ** -0.5
RG8 = [[0, 1, 2, 3, 4, 5, 6, 7]]

# head dealing (snake by per-head band cost) and per-slot band cutoffs
GROUP_HEADS = [[15, 8, 7, 0], [14, 9, 6, 1], [13, 10, 5, 2], [12, 11, 4, 3]]
SLOT_CUT = [256, 128, 128, 128]

NJ = N // 128               # 16 j-chunks
NI = N // 512               # 4 i-slices
EB_W = 2048                 # ebias table width; u = (i - j) + EB_OFF
EB_OFF = 1023

# d-permutation inside each head: [0, 32, 1, 33, ...] so rotate_half becomes
# an even/odd partition swap (StreamShuffle-able within 32-row quadrants).
D_PERM = [x for i in range(32) for x in (i, i + 32)]
SHUF_MASK = [x for i in range(16) for x in (2 * i + 1, 2 * i)]


def kept_j_chunks(cut, i0):
    return [j for j in range(NJ) if i0 - (j * 128 + 127) <= cut]


def build_program(dbg=False):
    nc = bacc.Bacc("TRN2", target_bir_lowering=False, debug=False,
                   num_devices=N_CORES)
    dbg_outs = {}

    def dbg_tensor(name, shape, dt_=F32):
        dbg_outs[name] = nc.dram_tensor(name, shape, dt_, kind="ExternalOutput")
        return dbg_outs[name]

    xT = nc.dram_tensor("xT", [C, N], BF, kind="ExternalInput")
    wT = nc.dram_tensor("wT", [C, 768], BF, kind="ExternalInput")
    ct = nc.dram_tensor("ct", [128, N], BF, kind="ExternalInput")
    st = nc.dram_tensor("st", [128, N], BF, kind="ExternalInput")
    ebias = nc.dram_tensor("ebias", [G, 128, EB_W], BF, kind="ExternalInput")
    pwT = nc.dram_tensor("pwT", [C, C], BF, kind="ExternalInput")
    pb = nc.dram_tensor("pb", [128, 8], F32, kind="ExternalInput")
    qidx = nc.dram_tensor("qidx", [128, 2], mybir.dt.uint32,
                          kind="ExternalInput")
    out = nc.dram_tensor("out", [C, 512], F32, kind="ExternalOutput")

    with tile.TileContext(nc) as tc:
        with tc.tile_pool(name="persist", bufs=1) as pp, \
             tc.tile_pool(name="work", bufs=1) as wp, \
             tc.tile_pool(name="psum", bufs=1, space="PSUM") as psp, \
             tc.tile_pool(name="dram", bufs=1, space="DRAM") as dp:

            # ---- persistent tiles ----
            qkp = [pp.tile([128, N], BF, name=f"qk{m}") for m in range(4)]
            vts = [pp.tile([128, 2 * G * 65], BF, name=f"vt{j}")
                   for j in range(NJ // 2)]

            def v_ap(j, slot):
                base = (j % 2) * G * 65 + slot * 65
                return vts[j // 2][:, base:base + 65]

            yts = [pp.tile([128, N], BF, name=f"yt{i}") for i in range(2)]
            pbt = pp.tile([128, 8], F32, name="pbt")
            qit = pp.tile([128, 2], mybir.dt.uint32, name="qit")
            # per-slot duplicated q (so QK's moving operand spans 128
            # partitions) and block-diag k stationaries (two 64-row j-chunks
            # of one head on the diagonal -> K=128 full-rate streaming)
            qds = [pp.tile([128, N], BF, name=f"qd{s}") for s in range(G)]
            bdb = [pp.tile([128, N], BF, name=f"bd{i}") for i in range(2)]
            nc.vector.memset(bdb[0][:], 0.0)
            nc.vector.memset(bdb[1][:], 0.0)

            # ---- input DMAs (interleaved so matmuls can start early) ----
            ph1 = tc.tile_pool(name="ph1", bufs=1)
            p1 = ph1.__enter__()
            xts, wts = [], []
            for ci in range(8):
                t = p1.tile([128, N], BF, name=f"xt{ci}")
                nc.sync.dma_start(t[:], xT.ap()[ci * 128:(ci + 1) * 128, :])
                xts.append(t)
                t = p1.tile([128, 768], BF, name=f"wt{ci}")
                nc.sync.dma_start(t[:], wT.ap()[ci * 128:(ci + 1) * 128, :])
                wts.append(t)
            ctt = p1.tile([128, N], BF, name="ctt")
            nc.sync.dma_start(ctt[:], ct.ap()[:, :])
            stt = p1.tile([128, N], BF, name="stt")
            nc.sync.dma_start(stt[:], st.ap()[:, :])
            ebs = []
            for s in range(G):
                t = pp.tile([128, EB_W], BF, name=f"eb{s}")
                nc.sync.dma_start(t[:], ebias.ap()[s, :, :])
                ebs.append(t)
            nc.sync.dma_start(pbt[:], pb.ap()[:, :])
            nc.sync.dma_start(qit[:], qidx.ap()[:, :])
            pwts = []
            for ci in range(8):
                t = pp.tile([128, C], BF, name=f"pwt{ci}")
                nc.sync.dma_start(t[:], pwT.ap()[ci * 128:(ci + 1) * 128, :])
                pwts.append(t)
            ot_acc = [pp.tile([128, 512], F32, name=f"oacc{co}")
                      for co in range(8)]

            # ---- building blocks ----
            def emit_bd(slot):
                # block-diag k stationary: two partition-shifting sbuf->sbuf
                # DMAs into a pre-zeroed buffer
                kh_ = qkp[2 + slot // 2][(slot % 2) * 64:
                                         (slot % 2) * 64 + 64, :]
                khv = kh_.rearrange("p (j c) -> p j c", c=128)
                bdv = bdb[slot % 2][:].rearrange("p (j c) -> p j c", c=128)
                nc.sync.dma_start(bdv[0:64, :, 0:64], khv[:, :, 0:64])
                nc.sync.dma_start(bdv[64:128, :, 64:128], khv[:, :, 64:128])

            def emit_qdup(m):
                for slot in (2 * m, 2 * m + 1):
                    src_ = qkp[m][(slot % 2) * 64:(slot % 2) * 64 + 64, :]
                    nc.sync.dma_start(qds[slot][0:64, :], src_)
                    nc.sync.dma_start(qds[slot][64:128, :], src_)

            def qkv_ip(m, ip):
                # one [128, 1024] psum group of q/k projection + RoPE
                # wT cols: q01 | q23 | k01 | k23 | v
                sl = slice(ip * 1024, (ip + 1) * 1024)
                ps = psp.tile([128, 1024], F32, name="qkvps", tag="s", bufs=2)
                for ci in range(8):
                    for hh in range(2):
                        hs = slice((2 * ip + hh) * 512,
                                   (2 * ip + hh + 1) * 512)
                        nc.tensor.matmul(
                            ps[:, hh * 512:(hh + 1) * 512],
                            wts[ci][:, m * 128:(m + 1) * 128],
                            xts[ci][:, hs],
                            start=(ci == 0), stop=(ci == 7))
                rot = wp.tile([128, 1024], F32, name="rot", tag="rot", bufs=1)
                nc.vector.stream_shuffle(rot[:], ps[:], SHUF_MASK)
                t1 = wp.tile([128, 1024], BF, name="ropet1", tag="ropet1",
                             bufs=1)
                nc.vector.tensor_mul(t1[:], rot[:], stt[:, sl])
                t2 = wp.tile([128, 1024], BF, name="ropet2", tag="ropet2",
                             bufs=1)
                nc.vector.tensor_mul(t2[:], ps[:], ctt[:, sl])
                nc.gpsimd.tensor_add(qkp[m][:, sl], t2[:], t1[:])

            def vdir_jp(jp):
                # v for j-chunks 2jp, 2jp+1 (direct, non-transposed)
                # shares the "s" psum tag: the oTp tags hold live attention
                # accumulators while vdir runs as filler inside attn0
                pv = psp.tile([128, 512], F32, name="vps", tag="s", bufs=2)
                for hh in range(2):
                    j = 2 * jp + hh
                    for ci in range(8):
                        nc.tensor.matmul(
                            pv[:, hh * 256:(hh + 1) * 256],
                            xts[ci][:, j * 128:(j + 1) * 128],
                            wts[ci][:, 512:768],
                            start=(ci == 0), stop=(ci == 7))
                vt_v = vts[jp][:].rearrange("p (a h e) -> p a h e", a=2, e=65)
                nc.vector.tensor_copy(
                    vt_v[:, :, :, 0:64],
                    pv[:].rearrange("p (a h e) -> p a h e", a=2, e=64))
                nc.vector.memset(vt_v[:, :, :, 64:65], 1.0)

            # ---- attention per head slot (j-outer) with PE fillers ----
            ag_outs = []
            ytf = [None] * 8

            def emit_gather(s):
                for hh in range(2):
                    t = pp.tile([128, 512], BF, name=f"ytf{s}_{hh}")
                    nc.gpsimd.indirect_dma_start(
                        out=t[:], out_offset=None, in_=ag_outs[s][:],
                        in_offset=bass.IndirectOffsetOnAxis(
                            ap=qit[:, hh:hh + 1], axis=0))
                    ytf[2 * s + hh] = t

            def emit_a2a(slot):
                # shard j = (slot-y, quarter j%4)
                ag_in = dp.tile([512, 512], BF, name=f"ag_in{slot}")
                ag_out = dp.tile([512, 512], BF, name=f"ag_out{slot}")
                ag_outs.append(ag_out)
                r0 = (slot % 2) * 64
                for shard in range(8):
                    nc.sync.dma_start(
                        ag_in[shard * 64:(shard + 1) * 64, :],
                        yts[slot // 2][r0:r0 + 64,
                                       (shard % 4) * 512:(shard % 4 + 1) * 512])
                with nc.named_scope(f"a2a{slot}"):
                    nc.gpsimd.collective_compute(
                        "AllToAll", mybir.AluOpType.bypass,
                        replica_groups=RG8,
                        ins=[ag_in.opt()], outs=[ag_out.opt()])

            def proj_pass1(co):
                # slots 0/1 contribution (ci 0..3) -> SBUF accumulator
                pj = psp.tile([128, 512], F32, name="pj", tag="s", bufs=2)
                for ci in range(4):
                    nc.tensor.matmul(pj[:],
                                     pwts[ci][:, co * 128:(co + 1) * 128],
                                     ytf[ci][:], start=(ci == 0),
                                     stop=(ci == 3))
                nc.vector.tensor_copy(ot_acc[co][:], pj[:])

            def proj_pass2(co):
                pj = psp.tile([128, 512], F32, name="pj", tag="s", bufs=2)
                for ci in range(4, 8):
                    nc.tensor.matmul(pj[:],
                                     pwts[ci][:, co * 128:(co + 1) * 128],
                                     ytf[ci][:], start=(ci == 4),
                                     stop=(ci == 7))
                tsum = wp.tile([128, 512], F32, name="tsum", tag="tsum",
                               bufs=2)
                nc.vector.tensor_add(tsum[:], pj[:], ot_acc[co][:])
                ot = wp.tile([128, 512], F32, name="ot", tag="ot", bufs=2)
                nc.scalar.add(ot[:], tsum[:], pbt[:, co:co + 1])
                nc.sync.dma_start(out.ap()[co * 128:(co + 1) * 128, :], ot[:])

            def attn_slot(slot, fillers):
                """j-outer attention for one slot; fillers: list of
                (after_j, fn) PE work interleaved into the stream."""
                cut = SLOT_CUT[slot]
                ebt = ebs[slot]
                bd = bdb[slot % 2]
                kept_per_isl = [kept_j_chunks(cut, isl * 512)
                                for isl in range(NI)]
                oTp = [psp.tile([65, 1024], F32, name=f"oTp{i}",
                                tag=f"oTp{i}", bufs=1,
                                padded_shape=[128, 1024])
                       for i in range(2)]
                oTs = [oTp[isl // 2][:, (isl % 2) * 512:(isl % 2) * 512 + 512]
                       for isl in range(NI)]
                fi = 0
                with nc.named_scope(f"attn{slot}"):
                    for j in range(NJ):
                        while fi < len(fillers) and fillers[fi][0] <= j:
                            fillers[fi][1]()
                            fi += 1
                        j0 = j * 128
                        isls = [isl for isl in range(NI)
                                if j in kept_per_isl[isl]]
                        groups = [isls[k:k + 2]
                                  for k in range(0, len(isls), 2)]
                        ps_list = []
                        for grp in groups:
                            w = 512 * len(grp)
                            i0 = grp[0] * 512
                            s = psp.tile([128, 1024], F32, name="s", tag="s",
                                         bufs=2)
                            for hh, isl in enumerate(grp):
                                nc.tensor.matmul(
                                    s[:, hh * 512:(hh + 1) * 512],
                                    bd[:, j0:j0 + 128],
                                    qds[slot][:, isl * 512:(isl + 1) * 512],
                                    start=True, stop=True)
                            p = wp.tile([128, 1024], BF, name="p", tag="p",
                                        bufs=4)
                            nc.scalar.activation(
                                p[:, 0:w], s[:, 0:w],
                                mybir.ActivationFunctionType.Exp, scale=SCALE)
                            if j0 - (grp[-1] * 512) < 512:
                                off = i0 - j0 + EB_OFF
                                nc.vector.tensor_mul(p[:, 0:w], p[:, 0:w],
                                                     ebt[:, off:off + w])
                            ps_list.append((grp, p))
                        for grp, p in ps_list:
                            for hh, isl in enumerate(grp):
                                j_kept = kept_per_isl[isl]
                                nc.tensor.matmul(
                                    oTs[isl], v_ap(j, slot),
                                    p[:, hh * 512:(hh + 1) * 512],
                                    start=(j == j_kept[0]),
                                    stop=(j == j_kept[-1]))
                    while fi < len(fillers):
                        fillers[fi][1]()
                        fi += 1
                    # normalize: y = oT[:64] / oT[64]
                    for pair in range(2):
                        i0 = pair * 1024
                        oT = oTp[pair]
                        den = wp.tile([1, 1024], F32, name="den", tag="den",
                                      bufs=1)
                        nc.scalar.copy(den[:], oT[64:65, :])
                        rec = wp.tile([1, 1024], F32, name="rec", tag="rec",
                                      bufs=1)
                        nc.vector.reciprocal_approx_fast(rec[:], den[:])
                        R = wp.tile([64, 1024], F32, name="R", tag="R",
                                    bufs=1)
                        nc.gpsimd.partition_broadcast(R[:], rec[:])
                        yt = yts[slot // 2]
                        r0 = (slot % 2) * 64
                        nc.vector.tensor_mul(yt[r0:r0 + 64, i0:i0 + 1024],
                                             oT[0:64, :], R[:])

            # ================= emission schedule =================
            sc = nc.enter_named_scope("qkv0", False)
            qkv_ip(2, 0)
            qkv_ip(2, 1)
            emit_bd(0)
            emit_bd(1)
            qkv_ip(0, 0)
            qkv_ip(0, 1)
            emit_qdup(0)
            nc.leave_named_scope("qkv0", sc[0], False)

            # attn0 with v-compute interleaved: AV(j) needs vts[j//2]
            attn_slot(0, [
                (0, lambda: vdir_jp(0)),
                (1, lambda: vdir_jp(1)),
                (3, lambda: vdir_jp(2)),
                (5, lambda: vdir_jp(3)),
                (7, lambda: vdir_jp(4)),
                (9, lambda: vdir_jp(5)),
                (11, lambda: vdir_jp(6)),
                (13, lambda: vdir_jp(7)),
            ])
            emit_a2a(0)

            # fillers: k23 (m=3) then q23 (m=1) for slots 2/3
            attn_slot(1, [
                (1, lambda: qkv_ip(3, 0)),
                (4, lambda: qkv_ip(3, 1)),
                (7, lambda: qkv_ip(1, 0)),
                (10, lambda: qkv_ip(1, 1)),
                (12, lambda: emit_qdup(1)),
                (12, lambda: emit_bd(2)),
            ])
            emit_a2a(1)

            attn_slot(2, [
                (1, lambda: emit_gather(0)),
                (3, lambda: emit_gather(1)),
                (5, lambda: proj_pass1(0)),
                (7, lambda: proj_pass1(1)),
                (9, lambda: proj_pass1(2)),
                (11, lambda: proj_pass1(3)),
                (13, lambda: emit_bd(3)),
            ])
            emit_a2a(2)

            attn_slot(3, [
                (5, lambda: proj_pass1(4)),
                (7, lambda: proj_pass1(5)),
                (9, lambda: proj_pass1(6)),
                (11, lambda: proj_pass1(7)),
                (13, lambda: emit_gather(2)),
            ])
            emit_a2a(3)
            emit_gather(3)

            if dbg:
                for m in range(4):
                    t = dbg_tensor(f"dbg_qk{m}", [128, N], BF)
                    nc.sync.dma_start(t.ap()[:, :], qkp[m][:])
                t = dbg_tensor("dbg_vt0", [128, 2 * G * 65], BF)
                nc.sync.dma_start(t.ap()[:, :], vts[0][:])
                for i in range(2):
                    t = dbg_tensor(f"dbg_yt{i}", [128, N], BF)
                    nc.sync.dma_start(t.ap()[:, :], yts[i][:])
                t = dbg_tensor("dbg_bd0", [128, N], BF)
                nc.sync.dma_start(t.ap()[:, :], bdb[0][:])
                t = dbg_tensor("dbg_qd0", [128, N], BF)
                nc.sync.dma_start(t.ap()[:, :], qds[0][:])
                for hh in range(2):
                    t = dbg_tensor(f"dbg_ytf{hh}", [128, 512], BF)
                    nc.sync.dma_start(t.ap()[:, :], ytf[hh][:])
                t = dbg_tensor("dbg_oacc0", [128, 512], F32)
                nc.sync.dma_start(t.ap()[:, :], ot_acc[0][:])

            sc = nc.enter_named_scope("proj", False)
            for co in range(8):
                proj_pass2(co)
            nc.leave_named_scope("proj", sc[0], False)
            ph1.__exit__(None, None, None)

    nc.compile()
    return nc


def prep_inputs(x, qkv_w, proj_w, proj_b, slopes):
    """Build the 8 per-core input maps (all host-side numpy)."""
    x = np.asarray(x, np.float32)
    qkv_w = np.asarray(qkv_w, np.float32)
    proj_w = np.asarray(proj_w, np.float32)
    proj_b = np.asarray(proj_b, np.float32)
    slopes = np.asarray(slopes, np.float32)

    # RoPE tables (transposed [d, n], d pairwise-interleaved, x2 head copies)
    inv = 1.0 / (10000.0 ** (np.arange(0, D, 2, dtype=np.float32) / D))
    fr = np.arange(N, dtype=np.float32)[:, None] * inv[None, :]   # [N, 32]
    sin_t, cos_t = np.sin(fr), np.cos(fr)
    ct64 = np.empty((64, N), np.float32)
    st64 = np.empty((64, N), np.float32)
    ct64[0::2] = cos_t.T
    ct64[1::2] = cos_t.T
    st64[0::2] = -sin_t.T
    st64[1::2] = sin_t.T
    ct = np.ascontiguousarray(np.vstack([ct64, ct64])).astype(NPBF)
    st = np.ascontiguousarray(np.vstack([st64, st64])).astype(NPBF)

    pos_p = np.arange(128, dtype=np.float64)[:, None]
    t_off = np.arange(EB_W, dtype=np.float64)[None, :] - EB_OFF
    dmin = np.minimum(pos_p - t_off, 0.0)  # j - i clipped

    in_maps = []
    for core in range(N_CORES):
        b = core // 4
        g = core % 4
        heads = GROUP_HEADS[g]
        rows = []
        for kind in range(2):  # q, k (d-permuted)
            for h in heads:
                base = kind * C + h * D
                rows.extend(base + p for p in D_PERM)
        for h in heads:        # v (natural d order)
            rows.extend(2 * C + h * D + d for d in range(D))
        wT_c = np.ascontiguousarray(qkv_w[rows, :].T)      # [1024, 768]

        eb_c = np.empty((G, 128, EB_W), np.float32)
        for s, h in enumerate(heads):
            eb_c[s] = np.exp(float(slopes[h]) * MAX_BIAS * dmin)

        # pwT rows (ci) ordered as the AG outputs: half h rows =
        # [rank0 (slot 2h, 2h+1), rank1, ..., rank3] x 64 d each.
        pwT_c = np.empty((C, C), np.float32)
        for s in range(4):
            for rank in range(4):
                hh = GROUP_HEADS[rank][s]
                r = 256 * s + 64 * rank
                pwT_c[r:r + 64, :] = proj_w[:, hh * D:(hh + 1) * D].T
        pb_c = np.ascontiguousarray(proj_b.reshape(8, 128).T)

        # quarter-gather row indices into ag_out viewed as [(r q) w]
        qidx_c = np.empty((128, 2), np.uint32)
        p_ = np.arange(128)
        for hh in range(2):
            rank = 2 * hh + p_ // 64
            qidx_c[:, hh] = 64 * (4 * b + rank) + p_ % 64

        in_maps.append({
            "xT": np.ascontiguousarray(x[b].T).astype(NPBF),
            "wT": wT_c.astype(NPBF),
            "ct": ct, "st": st,
            "ebias": eb_c.astype(NPBF),
            "pwT": pwT_c.astype(NPBF),
            "pb": pb_c,
            "qidx": qidx_c,
        })
    return in_maps


_NC = None


def _get_nc():
    global _NC
    if _NC is None:
        _NC = build_program()
    return _NC


def run(inputs, trace=False):
    nc = _get_nc()
    in_maps = prep_inputs(**inputs)
    res = run_bass_kernel_spmd(nc, in_maps, core_ids=list(range(N_CORES)),
                               trace=trace)
    out = np.empty((B, N, C), np.float32)
    for core in range(N_CORES):
        b, g = core // 4, core % 4
        out[b, g * 512:(g + 1) * 512, :] = res.results[core]["out"].T
    return out, res


def kernel(**inputs) -> np.ndarray:
    out, _ = run(inputs, trace=False)
    return out
